# revision 1
# baseline (speedup 1.0000x reference)
"""Trainium2 Bass kernel for a steerable group-CNN (GCNN) forward pass.

Pipeline (per image):
  conv1: 1->128 ch, 3x3, pad 1   (rotated-kernel construction done on host)
  relu
  conv2: 128->256 ch, 3x3, pad 1 (circulant group weight, built on host)
  relu
  group-pool: mean over inner-8 channel factor -> 32 ch
  fc: (32*28*28) -> 10

Device strategy (pure data parallel, batch 512 / 8 cores = 64 images/core):
  - conv1 as a single K=9 matmul per half image (im2col of x built on host):
      out[oc, pix] = sum_tap w1c[tap, oc] * x9[tap, pix]
    -> h laid out channels-on-partitions, pixels-free.  ReLU'd into a
    zero-padded 30x30 SBUF image (hpad) so conv2 can read shifted windows.
  - conv2 with *shifted activations as the stationary operand*:
      out[(y,x), oc] += hpad[:, y+dy, x+dx].T @ wt[:, (dy,dx), :]
    9 accumulating matmuls per 4-row chunk (M=112 pixels, N=256 channels).
    Output lands pixels-on-partitions / channels-free, which makes the
    group-pool a free-dim strided reduce on VectorE.
  - pool+fc folded: p = reduce_add over inner-8 channels; the 1/8 mean and
    the flatten order are folded into a host-rearranged fc weight. FC is a
    pixel-contraction matmul accumulated over (chunk, group) into one PSUM.
"""

import os

import numpy as np

import concourse.tile as tile
from concourse import bacc, mybir
from concourse.bass_utils import run_bass_kernel_spmd

G = 8
KS = 3
HW = 28
PW = HW + 2          # padded image width
NPIX = HW * HW       # 784
NCH1 = 128           # conv1 out channels (G*16)
NCH2 = 256           # conv2 out channels (G*32)
NA = 32              # pooled channels
NCLS = 10
# conv2 processes M=128 contiguous *padded* (30-wide) flat positions per chunk:
# chunk c covers padded-flat positions f = 128c + m.  Valid output pixels are
# f = y*30+x with x<28, y<28; everything else is a junk partition annihilated
# by zero rows in the host-built fc weight.
PIXM = 128
NCHK = -(-(HW * PW - 2) // PIXM)   # 7 chunks cover flat [0, 838)
# taps read hp[128c + m + dy*30 + dx]; max = 128*6 + 127 + 62 = 957
HP_LEN = 960
N_CORES = 8
B_TOT = 512
B_LOC = B_TOT // N_CORES      # 64
C_IMG = 8                     # images per x9 DMA chunk

# matmul operand dtype for conv2 (h storage + group weight): "f32", "f32r", "bf16"
CONV_DT = os.environ.get("GCNN_CONV_DT", "bf16")
# conv1 operand dtype (x im2col + rotated base weight); f32r costs the same
# cycles as bf16 here (N=392 >= 256) but keeps the input un-quantized.
# "mixed" = bf16 stationary weight (fast load) + f32r moving input.
C1_DT = os.environ.get("GCNN_C1_DT", "f32r")
# fc/pool dtype: "f32", "f32r", or "bf16"
FC_DT = os.environ.get("GCNN_FC_DT", "f32r")

_F32 = mybir.dt.float32
_BF16 = mybir.dt.bfloat16
_F32R = mybir.dt.float32r


def _store_dt(kind):
    if kind == "bf16":
        return _BF16
    if kind == "f32r":
        return _F32R
    return _F32


def _np_dt(kind):
    import ml_dtypes
    return ml_dtypes.bfloat16 if kind == "bf16" else np.float32


def _mm(ap, kind):
    return ap


# ---------------------------------------------------------------------------
# Host-side weight construction (replicates the reference's jax math in numpy)
# ---------------------------------------------------------------------------

def _bilinear_sample(img, px, py):
    K = img.shape[-1]
    x0 = np.floor(px)
    y0 = np.floor(py)
    wx = (px - x0).astype(np.float32)
    wy = (py - y0).astype(np.float32)
    x0i = x0.astype(np.int32)
    y0i = y0.astype(np.int32)

    def gather(yi, xi):
        valid = (yi >= 0) & (yi < K) & (xi >= 0) & (xi < K)
        yc = np.clip(yi, 0, K - 1)
        xc = np.clip(xi, 0, K - 1)
        return img[:, :, yc, xc] * valid.astype(img.dtype)

    return (gather(y0i, x0i) * (1 - wx) * (1 - wy)
            + gather(y0i, x0i + 1) * wx * (1 - wy)
            + gather(y0i + 1, x0i) * (1 - wx) * wy
            + gather(y0i + 1, x0i + 1) * wx * wy)


def _rotated_kernels(base, group_order):
    K = base.shape[-1]
    coords = ((2.0 * np.arange(K, dtype=np.float32) + 1.0) / K - 1.0).astype(np.float32)
    xs, ys = np.meshgrid(coords, coords, indexing="xy")
    out = np.empty((group_order,) + base.shape, np.float32)
    for k in range(group_order):
        theta = np.float32(2.0 * np.pi * k / group_order)
        c, s = np.float32(np.cos(theta)), np.float32(np.sin(theta))
        gx = c * xs - s * ys
        gy = s * xs + c * ys
        px = ((gx + 1.0) * K - 1.0) / 2.0
        py = ((gy + 1.0) * K - 1.0) / 2.0
        out[k] = _bilinear_sample(base, px.astype(np.float32), py.astype(np.float32))
    return out


def _host_prep(x, base_weight, w2, fc_w, fc_b):
    conv_np = _np_dt(CONV_DT)
    c1_np = _np_dt("f32r" if C1_DT == "mixed" else C1_DT)
    w1_np = _np_dt("bf16" if C1_DT == "mixed" else C1_DT)
    fc_np = _np_dt(FC_DT)

    rk = _rotated_kernels(base_weight.astype(np.float32), G)   # (G, 16, 1, 3, 3)
    w1 = rk.reshape(G * 16, 1, KS, KS)                         # (128, 1, 3, 3)
    w1c = np.ascontiguousarray(w1[:, 0].reshape(NCH1, 9).T)    # (9, 128), tap=dy*3+dx

    gi = np.arange(G)[:, None]
    hi = np.arange(G)[None, :]
    idx = (gi - hi) % G
    Wc = w2[:, :, idx]                                          # (32, 16, G, G, 3, 3)
    Wbig = np.transpose(Wc, (2, 0, 1, 3, 4, 5)).reshape(NCH2, NCH1, KS, KS)
    # wt[ic, tap, oc] = Wbig[oc, ic, dy, dx]
    wt = np.ascontiguousarray(np.transpose(Wbig, (1, 2, 3, 0))).reshape(NCH1, 9 * NCH2)

    # fcw[m, c, a*10+n] = fc_w[n, a*784 + y*28 + x] / 8 for f = 128c+m = y*30+x
    # when (y, x) is a real pixel; zero for junk positions.
    f8 = (fc_w.astype(np.float64) / 8.0).astype(np.float32).reshape(NCLS, NA, HW, HW)
    fcw = np.zeros((PIXM, NCHK, NA, NCLS), np.float32)
    for c in range(NCHK):
        for m in range(PIXM):
            ff = c * PIXM + m
            yy, xx = ff // PW, ff % PW
            if yy < HW and xx < HW:
                fcw[m, c] = f8[:, :, yy, xx].T
    fcw = np.ascontiguousarray(fcw.reshape(PIXM, NCHK * NA * NCLS))

    # im2col of padded x: x9[tap, b, pix] = xpad[b, y+dy, x+dx]
    B = x.shape[0]
    xp = np.zeros((B, PW, PW), np.float32)
    xp[:, 1:1 + HW, 1:1 + HW] = x[:, 0]
    x9 = np.empty((9, B, HW, HW), np.float32)
    for dy in range(3):
        for dx in range(3):
            x9[dy * 3 + dx] = xp[:, dy:dy + HW, dx:dx + HW]
    x9 = x9.reshape(9, B, NPIX)

    return {
        "x9": np.ascontiguousarray(x9.astype(c1_np)),
        "w1c": np.ascontiguousarray(w1c.astype(w1_np)),
        "wt": np.ascontiguousarray(wt.astype(conv_np)),
        "fcw": np.ascontiguousarray(fcw.astype(fc_np)),
        "fcb": np.ascontiguousarray(fc_b.reshape(NCLS, 1).astype(np.float32)),
    }


# ---------------------------------------------------------------------------
# Device kernel
# ---------------------------------------------------------------------------

def build_bass():
    from contextlib import ExitStack

    conv_sdt = _store_dt(CONV_DT)
    c1_sdt = _store_dt("f32r" if C1_DT == "mixed" else C1_DT)
    w1_sdt = _store_dt("bf16" if C1_DT == "mixed" else C1_DT)
    fc_sdt = _store_dt(FC_DT)

    nc = bacc.Bacc()
    x9_d = nc.declare_dram_parameter("x9", [9, B_LOC, NPIX], c1_sdt, isOutput=False)
    w1c_d = nc.declare_dram_parameter("w1c", [9, NCH1], w1_sdt, isOutput=False)
    wt_d = nc.declare_dram_parameter("wt", [NCH1, 9 * NCH2], conv_sdt, isOutput=False)
    fcw_d = nc.declare_dram_parameter("fcw", [PIXM, NCHK * NA * NCLS], fc_sdt,
                                      isOutput=False)
    fcb_d = nc.declare_dram_parameter("fcb", [NCLS, 1], _F32, isOutput=False)
    out_d = nc.declare_dram_parameter("out", [B_LOC, NCLS], _F32, isOutput=True)

    with tile.TileContext(nc) as tc, ExitStack() as ctx:
        consts = ctx.enter_context(tc.tile_pool(name="consts", bufs=1))
        x9_pool = ctx.enter_context(tc.tile_pool(name="x9", bufs=2))
        hp_pool = ctx.enter_context(tc.tile_pool(name="hpad", bufs=4))
        h2_pool = ctx.enter_context(tc.tile_pool(name="h2r", bufs=4))
        ps1_pool = ctx.enter_context(tc.tile_pool(name="ps1", bufs=2, space="PSUM"))
        ps2_pool = ctx.enter_context(tc.tile_pool(name="ps2", bufs=4, space="PSUM"))
        psfc_pool = ctx.enter_context(tc.tile_pool(name="psfc", bufs=1, space="PSUM"))

        # First input chunk: tiny (2 images) and issued before everything else
        # so conv1 can start as early as possible.
        x9_first = consts.tile([9, 2, NPIX], c1_sdt)
        nc.sync.dma_start(x9_first[:], x9_d[:, 0:2, :])
        w1c_t = consts.tile([9, NCH1], w1_sdt)
        nc.sync.dma_start(w1c_t[:], w1c_d[:])

        # PE warm-up: dependency-free matmuls keep the tensor engine busy from
        # engine start, flipping the HAM clock gate to 2.4 GHz before the real
        # work arrives and hiding the initial weight/input DMA latency.
        warm_pool = ctx.enter_context(tc.tile_pool(name="warm", bufs=1, space="PSUM"))
        warm_sb = consts.tile([NCH1, 512], conv_sdt)
        nc.vector.memset(warm_sb[:].bitcast(_F32) if conv_sdt == _F32R else warm_sb[:],
                         0.125)
        warm_ps = warm_pool.tile([NCH1, 512], _F32)
        for _ in range(10):
            nc.tensor.matmul(warm_ps[:], lhsT=warm_sb[:, :NCH1], rhs=warm_sb[:],
                             start=True, stop=True)

        # resident tensors
        wt_t = consts.tile([NCH1, 9, NCH2], conv_sdt)
        nc.sync.dma_start(wt_t[:], wt_d[:].rearrange("p (t o) -> p t o", o=NCH2))
        fcb_t = consts.tile([NCLS, 1], _F32)
        nc.sync.dma_start(fcb_t[:], fcb_d[:])
        # fcw is only needed by the fc tail; load it off the critical start path
        fcw_t = consts.tile([PIXM, NCHK, NA * NCLS], fc_sdt)
        # pooled activations for the whole local batch: p_all[pix, c, b, a]
        p_all = consts.tile([PIXM, NCHK, B_LOC, NA], fc_sdt)

        half = NPIX // 2  # 392

        def _ms(ap):
            # memset rejects float32r; zero bits through a float32 view
            nc.gpsimd.memset(ap.bitcast(_F32) if conv_sdt == _F32R else ap, 0.0)

        def conv1(b, x9_t, bi):
            """h(b) = relu(conv1(x(b))) written into a padded 30x30 image."""
            hp = hp_pool.tile([NCH1, HP_LEN], conv_sdt, tag="hp")
            hp3 = hp[:, :PW * PW].rearrange("p (y x) -> p y x", x=PW)
            # zero the 1-pixel border (interior is fully overwritten below)
            _ms(hp3[:, 0, :])
            _ms(hp3[:, PW - 1, :])
            _ms(hp3[:, 1:PW - 1, 0])
            _ms(hp3[:, 1:PW - 1, PW - 1])
            _ms(hp[:, PW * PW:])
            for h in range(2):
                ps1 = ps1_pool.tile([NCH1, half], _F32, tag="ps1")
                nc.tensor.matmul(
                    ps1[:],
                    lhsT=_mm(w1c_t[:], C1_DT),
                    rhs=_mm(x9_t[:, bi, h * half:(h + 1) * half], C1_DT),
                    start=True, stop=True,
                )
                # relu + downcast into hpad interior rows 14h..14h+13
                dst = hp3[:, 1 + 14 * h:1 + 14 * (h + 1), 1:1 + HW]
                nc.scalar.activation(
                    dst, ps1[:].rearrange("p (y x) -> p y x", x=HW),
                    mybir.ActivationFunctionType.Relu,
                )
            return hp

        def conv2(b, hp):
            """h2(b) -> relu -> group-pool into p_all[:, :, b, :].

            Chunks are processed in pairs sharing one PSUM bank (2 x 256 f32
            = one 2KB bank): the pair's first matmul start=True zeroes the
            whole bank; the second chunk's matmuls rely on pending-zero for
            their first write.  Halves the sem-inc rounds on PE and the
            relu/pool op counts."""
            for ci, cs in enumerate([(0, 1), (2, 3), (4, 5), (6,)]):
                nc2 = len(cs) * NCH2
                ps2 = ps2_pool.tile([PIXM, nc2], _F32, tag="ps2")
                nmm = len(cs) * 9
                i = 0
                for k, c in enumerate(cs):
                    for tap in range(9):
                        dy, dx = tap // 3, tap % 3
                        off = PIXM * c + dy * PW + dx
                        lhsT = hp[:, off:off + PIXM]
                        nc.tensor.matmul(
                            ps2[:, k * NCH2:(k + 1) * NCH2],
                            lhsT=_mm(lhsT, CONV_DT),
                            rhs=_mm(wt_t[:, tap, :], CONV_DT),
                            start=(i == 0), stop=(i == nmm - 1),
                        )
                        i += 1
                h2r = h2_pool.tile([PIXM, nc2], _F32, tag="h2r")
                nc.scalar.activation(h2r[:], ps2[:],
                                     mybir.ActivationFunctionType.Relu)
                with nc.allow_low_precision(reason="pool sum feeds reduced-precision fc"):
                    nc.vector.tensor_reduce(
                        p_all[:, cs[0]:cs[0] + len(cs), b, :],
                        h2r[:].rearrange("p (c a k) -> p c a k", k=G, a=NA),
                        axis=mybir.AxisListType.X,
                        op=mybir.AluOpType.add,
                    )

        # software-pipelined main loop (2-deep: conv1 runs 2 images ahead of
        # conv2 so startup relu latency never stalls the PE); images 0-1 come
        # from the early x9_first chunk
        DEPTH = 2
        hps = {}
        x9_t, x0, sz = x9_first, 0, 2
        for b in range(B_LOC + DEPTH):
            if b < B_LOC:
                if b == x0 + sz:
                    x0, sz = b, min(C_IMG, B_LOC - b)
                    x9_t = x9_pool.tile([9, sz, NPIX], c1_sdt, tag="x9")
                    nc.sync.dma_start(x9_t[:], x9_d[:, x0:x0 + sz, :])
                hps[b] = conv1(b, x9_t, b - x0)
            if b >= DEPTH:
                conv2(b - DEPTH, hps.pop(b - DEPTH))

        nc.sync.dma_start(fcw_t[:], fcw_d[:].rearrange("p (c m) -> p c m", m=NA * NCLS))

        # fc: out[n, bb] += fcw[:, c, a*10+n].T @ p_all[:, c, :, a]
        fc_ps = psfc_pool.tile([NCLS, B_LOC], _F32)
        nmm = NCHK * NA
        i = 0
        for c in range(NCHK):
            for a in range(NA):
                nc.tensor.matmul(
                    fc_ps[:],
                    lhsT=_mm(fcw_t[:, c, a * NCLS:(a + 1) * NCLS], FC_DT),
                    rhs=_mm(p_all[:, c, :, a], FC_DT),
                    start=(i == 0), stop=(i == nmm - 1),
                )
                i += 1
        out_sb = consts.tile([NCLS, B_LOC], _F32)
        nc.vector.tensor_scalar_add(out_sb[:], fc_ps[:], fcb_t[:])
        nc.sync.dma_start(out_d[:].rearrange("b n -> n b"), out_sb[:])

    if not nc.is_finalized():
        nc.finalize()
    return nc


_NC_CACHE = {}


def _get_nc():
    key = (CONV_DT, FC_DT)
    if key not in _NC_CACHE:
        _NC_CACHE[key] = build_bass()
    return _NC_CACHE[key]


def _run(x, base_weight, w2, fc_w, fc_b, **spmd_kwargs):
    x = np.asarray(x, np.float32)
    base_weight = np.asarray(base_weight, np.float32)
    w2 = np.asarray(w2, np.float32)
    fc_w = np.asarray(fc_w, np.float32)
    fc_b = np.asarray(fc_b, np.float32)

    prep = _host_prep(x, base_weight, w2, fc_w, fc_b)
    nc = _get_nc()
    in_maps = []
    for i in range(N_CORES):
        m = dict(prep)
        m["x9"] = np.ascontiguousarray(prep["x9"][:, i * B_LOC:(i + 1) * B_LOC, :])
        in_maps.append(m)
    res = run_bass_kernel_spmd(nc, in_maps, list(range(N_CORES)), **spmd_kwargs)
    out = np.concatenate([res.results[i]["out"] for i in range(N_CORES)], axis=0)
    return out, res


def kernel(x, base_weight, w2, fc_w, fc_b):
    out, _ = _run(x, base_weight, w2, fc_w, fc_b)
    return out



# revision 2
# speedup vs baseline: 1.1514x; 1.1514x over previous
"""Trainium2 Bass kernel for a steerable group-CNN (GCNN) forward pass.

Pipeline (per image):
  conv1: 1->128 ch, 3x3, pad 1   (rotated-kernel construction done on host)
  relu
  conv2: 128->256 ch, 3x3, pad 1 (circulant group weight, built on host)
  relu
  group-pool: mean over inner-8 channel factor -> 32 ch
  fc: (32*28*28) -> 10

Device strategy (pure data parallel, batch 512 / 8 cores = 64 images/core):
  - conv1 as a single K=9 matmul per half image (im2col of x built on host):
      out[oc, pix] = sum_tap w1c[tap, oc] * x9[tap, pix]
    -> h laid out channels-on-partitions, relu'd into a zero-padded 30x30
    SBUF image (hpad, bf16) so conv2 can read shifted windows.
  - conv2 FLIPPED vs the obvious layout: the *weights* are the stationary
    operand (reused across both 392-pixel halves -> LDWEIGHTS amortized and
    hidden by the PE reorder window), activations stream as the moving
    operand through 2D shifted-window APs over hpad:
      psum[oc_blk, (y,x)] += wt[:, tap, oc_blk].T @ hp[:, y+dy, x+dx]
    Streams exactly 784 real pixels per tap (no padded-junk columns), i.e.
    9*2*784 = 14112 column-cycles/image vs 16128 for the chunked layout.
  - psum -> relu -> h2 [128oc, 800] bf16; DVE 32x32 block-transpose gives
    h2T[32p+r, 32k+c] = h2[32p+c, 32k+r]; the group-pool is then a free-dim
    segmented reduce (DVE) over 8 consecutive channels.
  - fc consumes the block-transposed pooled layout directly: the host
    rearranges fc_w to match (any consistent (partition, free) indexing of
    the contraction works), 200 accumulating matmuls of N=64 images.
"""

import os

import numpy as np

import concourse.tile as tile
from concourse import bacc, mybir
from concourse.bass_utils import run_bass_kernel_spmd

G = 8
KS = 3
HW = 28
PW = HW + 2          # padded image width
NPIX = HW * HW       # 784
NCH1 = 128           # conv1 out channels (G*16)
NCH2 = 256           # conv2 out channels (G*32)
NCLS = 10
HP_LEN = 960
N_CORES = 8
B_TOT = 512
B_LOC = B_TOT // N_CORES      # 64
C_IMG = 8                     # images per x9 DMA chunk

PIXP = 800                    # h2 pixel dim padded to a multiple of 32
KB = PIXP // 32               # 25 transpose blocks
NGRP = 4                      # pool groups per 32-channel transpose block

# kept for test.py's config print
CONV_DT = "bf16"
FC_DT = "bf16"

_F32 = mybir.dt.float32
_BF16 = mybir.dt.bfloat16


# ---------------------------------------------------------------------------
# Host-side weight construction (replicates the reference's jax math in numpy)
# ---------------------------------------------------------------------------

def _bilinear_sample(img, px, py):
    K = img.shape[-1]
    x0 = np.floor(px)
    y0 = np.floor(py)
    wx = (px - x0).astype(np.float32)
    wy = (py - y0).astype(np.float32)
    x0i = x0.astype(np.int32)
    y0i = y0.astype(np.int32)

    def gather(yi, xi):
        valid = (yi >= 0) & (yi < K) & (xi >= 0) & (xi < K)
        yc = np.clip(yi, 0, K - 1)
        xc = np.clip(xi, 0, K - 1)
        return img[:, :, yc, xc] * valid.astype(img.dtype)

    return (gather(y0i, x0i) * (1 - wx) * (1 - wy)
            + gather(y0i, x0i + 1) * wx * (1 - wy)
            + gather(y0i + 1, x0i) * (1 - wx) * wy
            + gather(y0i + 1, x0i + 1) * wx * wy)


def _rotated_kernels(base, group_order):
    K = base.shape[-1]
    coords = ((2.0 * np.arange(K, dtype=np.float32) + 1.0) / K - 1.0).astype(np.float32)
    xs, ys = np.meshgrid(coords, coords, indexing="xy")
    out = np.empty((group_order,) + base.shape, np.float32)
    for k in range(group_order):
        theta = np.float32(2.0 * np.pi * k / group_order)
        c, s = np.float32(np.cos(theta)), np.float32(np.sin(theta))
        gx = c * xs - s * ys
        gy = s * xs + c * ys
        px = ((gx + 1.0) * K - 1.0) / 2.0
        py = ((gy + 1.0) * K - 1.0) / 2.0
        out[k] = _bilinear_sample(base, px.astype(np.float32), py.astype(np.float32))
    return out


def _host_prep(x, base_weight, w2, fc_w, fc_b):
    import ml_dtypes
    bf16 = ml_dtypes.bfloat16

    rk = _rotated_kernels(base_weight.astype(np.float32), G)   # (G, 16, 1, 3, 3)
    w1 = rk.reshape(G * 16, 1, KS, KS)                         # (128, 1, 3, 3)
    w1c = np.ascontiguousarray(w1[:, 0].reshape(NCH1, 9).T)    # (9, 128), tap=dy*3+dx

    gi = np.arange(G)[:, None]
    hi = np.arange(G)[None, :]
    idx = (gi - hi) % G
    Wc = w2[:, :, idx]                                          # (32, 16, G, G, 3, 3)
    Wbig = np.transpose(Wc, (2, 0, 1, 3, 4, 5)).reshape(NCH2, NCH1, KS, KS)
    # wt[ic, tap, oc] = Wbig[oc, ic, dy, dx]
    wt = np.ascontiguousarray(np.transpose(Wbig, (1, 2, 3, 0))).reshape(NCH1, 9 * NCH2)

    # fc weight rearranged for the block-transposed pooled layout:
    # fcw[q=32p+r, ocb, k, g, n] = fc_w[n, i*784 + pix] / 8
    #   with i = ocb*16 + 4p + g, pix = 32k + r  (zero for pix >= 784)
    f8 = (fc_w.astype(np.float64) / 8.0).astype(np.float32).reshape(NCLS, 32, NPIX)
    fcw = np.zeros((128, 2, KB, NGRP, NCLS), np.float32)
    for p in range(4):
        for r in range(32):
            q = 32 * p + r
            for k in range(KB):
                pix = 32 * k + r
                if pix >= NPIX:
                    continue
                for ocb in range(2):
                    for g in range(NGRP):
                        i = ocb * 16 + 4 * p + g
                        fcw[q, ocb, k, g] = f8[:, i, pix]
    fcw = np.ascontiguousarray(fcw.reshape(128, 2 * KB * NGRP * NCLS))

    # im2col of padded x: x9[tap, b, pix] = xpad[b, y+dy, x+dx]
    B = x.shape[0]
    xp = np.zeros((B, PW, PW), np.float32)
    xp[:, 1:1 + HW, 1:1 + HW] = x[:, 0]
    x9 = np.empty((9, B, HW, HW), np.float32)
    for dy in range(3):
        for dx in range(3):
            x9[dy * 3 + dx] = xp[:, dy:dy + HW, dx:dx + HW]
    x9 = x9.reshape(9, B, NPIX)

    return {
        "x9": np.ascontiguousarray(x9.astype(bf16)),
        "w1c": np.ascontiguousarray(w1c.astype(bf16)),
        "wt": np.ascontiguousarray(wt.astype(bf16)),
        "fcw": np.ascontiguousarray(fcw.astype(bf16)),
        "fcb": np.ascontiguousarray(fc_b.reshape(NCLS, 1).astype(np.float32)),
    }


# ---------------------------------------------------------------------------
# Device kernel
# ---------------------------------------------------------------------------

def build_bass():
    from contextlib import ExitStack

    nc = bacc.Bacc()
    x9_d = nc.declare_dram_parameter("x9", [9, B_LOC, NPIX], _BF16, isOutput=False)
    w1c_d = nc.declare_dram_parameter("w1c", [9, NCH1], _BF16, isOutput=False)
    wt_d = nc.declare_dram_parameter("wt", [NCH1, 9 * NCH2], _BF16, isOutput=False)
    fcw_d = nc.declare_dram_parameter("fcw", [128, 2 * KB * NGRP * NCLS], _BF16,
                                      isOutput=False)
    fcb_d = nc.declare_dram_parameter("fcb", [NCLS, 1], _F32, isOutput=False)
    out_d = nc.declare_dram_parameter("out", [B_LOC, NCLS], _F32, isOutput=True)

    with tile.TileContext(nc) as tc, ExitStack() as ctx:
        consts = ctx.enter_context(tc.tile_pool(name="consts", bufs=1))
        x9_pool = ctx.enter_context(tc.tile_pool(name="x9", bufs=2))
        hp_pool = ctx.enter_context(tc.tile_pool(name="hpad", bufs=4))
        h2_pool = ctx.enter_context(tc.tile_pool(name="h2", bufs=2))
        h2t_pool = ctx.enter_context(tc.tile_pool(name="h2t", bufs=2))
        ps1_pool = ctx.enter_context(tc.tile_pool(name="ps1", bufs=2, space="PSUM"))
        ps2_pool = ctx.enter_context(tc.tile_pool(name="ps2", bufs=2, space="PSUM"))
        psfc_pool = ctx.enter_context(tc.tile_pool(name="psfc", bufs=1, space="PSUM"))

        # First input chunk: tiny (2 images) and issued before everything else
        # so conv1 can start as early as possible.
        x9_first = consts.tile([9, 2, NPIX], _BF16)
        nc.sync.dma_start(x9_first[:], x9_d[:, 0:2, :])
        w1c_t = consts.tile([9, NCH1], _BF16)
        nc.sync.dma_start(w1c_t[:], w1c_d[:])

        # PE warm-up: dependency-free matmuls keep the tensor engine busy from
        # engine start, flipping the HAM clock gate to 2.4 GHz before the real
        # work arrives and hiding the initial weight/input DMA latency.
        warm_pool = ctx.enter_context(tc.tile_pool(name="warm", bufs=1, space="PSUM"))
        warm_sb = consts.tile([NCH1, 512], _BF16)
        nc.vector.memset(warm_sb[:], 0.125)
        warm_ps = warm_pool.tile([NCH1, 512], _F32)
        for _ in range(10):
            nc.tensor.matmul(warm_ps[:], lhsT=warm_sb[:, :NCH1], rhs=warm_sb[:],
                             start=True, stop=True)

        # resident tensors
        wt_t = consts.tile([NCH1, 9, NCH2], _BF16)
        nc.sync.dma_start(wt_t[:], wt_d[:].rearrange("p (t o) -> p t o", o=NCH2))
        fcb_t = consts.tile([NCLS, 1], _F32)
        nc.sync.dma_start(fcb_t[:], fcb_d[:])
        # fcw is only needed by the fc tail; load it off the critical start path
        fcw_t = consts.tile([128, 2, KB, NGRP, NCLS], _BF16)
        # pooled transposed activations for the whole local batch
        pT_all = consts.tile([128, 2, KB, NGRP, B_LOC], _BF16)

        half = NPIX // 2  # 392

        def conv1(b, x9_t, bi):
            """h(b) = relu(conv1(x(b))) written into a padded 30x30 image."""
            hp = hp_pool.tile([NCH1, HP_LEN], _BF16, tag="hp")
            hp3 = hp[:, :PW * PW].rearrange("p (y x) -> p y x", x=PW)
            # zero the 1-pixel border (interior is fully overwritten below)
            nc.gpsimd.memset(hp3[:, 0, :], 0.0)
            nc.gpsimd.memset(hp3[:, PW - 1, :], 0.0)
            nc.gpsimd.memset(hp3[:, 1:PW - 1, 0], 0.0)
            nc.gpsimd.memset(hp3[:, 1:PW - 1, PW - 1], 0.0)
            for h in range(2):
                ps1 = ps1_pool.tile([NCH1, half], _F32, tag="ps1")
                nc.tensor.matmul(
                    ps1[:],
                    lhsT=w1c_t[:],
                    rhs=x9_t[:, bi, h * half:(h + 1) * half],
                    start=True, stop=True,
                )
                # relu + downcast into hpad interior rows 14h..14h+13
                dst = hp3[:, 1 + 14 * h:1 + 14 * (h + 1), 1:1 + HW]
                nc.scalar.activation(
                    dst, ps1[:].rearrange("p (y x) -> p y x", x=HW),
                    mybir.ActivationFunctionType.Relu,
                )
            return hp

        def conv2(b, hp):
            """h2(b) -> relu -> transpose -> group-pool into pT_all[..., b]."""
            hp3 = hp[:, :PW * PW].rearrange("p (y x) -> p y x", x=PW)
            for ocb in range(2):
                # psum [128, 1024]: two 392-pixel halves at free offsets 0, 512
                # so each matmul output stays inside one 2KB psum bank.
                ps2 = ps2_pool.tile([128, 1024], _F32, tag="ps2")
                for tap in range(9):
                    dy, dx = tap // 3, tap % 3
                    lhsT = wt_t[:, tap, ocb * 128:(ocb + 1) * 128]
                    for h in range(2):
                        rhs = hp3[:, dy + 14 * h: dy + 14 * h + 14, dx: dx + HW]
                        nc.tensor.matmul(
                            ps2[:, 512 * h: 512 * h + half],
                            lhsT=lhsT, rhs=rhs,
                            start=(tap == 0), stop=(tap == 8),
                        )
                h2 = h2_pool.tile([128, PIXP], _BF16, tag="h2")
                nc.scalar.activation(
                    h2[:, :NPIX].rearrange("p (h f) -> p h f", h=2),
                    ps2[:].rearrange("p (h f) -> p h f", h=2)[:, :, :half],
                    mybir.ActivationFunctionType.Relu,
                )
                nc.gpsimd.memset(h2[:, NPIX:PIXP], 0.0)
                h2t = h2t_pool.tile([128, PIXP], _BF16, tag="h2t")
                nc.vector.transpose(h2t[:], h2[:])
                with nc.allow_low_precision(reason="pool sum feeds bf16 fc"):
                    nc.vector.tensor_reduce(
                        pT_all[:, ocb, :, :, b],
                        h2t[:].rearrange("p (k g j) -> p k g j", g=NGRP, j=G),
                        axis=mybir.AxisListType.X,
                        op=mybir.AluOpType.add,
                    )

        # software-pipelined main loop (2-deep: conv1 runs 2 images ahead of
        # conv2); images 0-1 come from the early x9_first chunk
        DEPTH = 2
        hps = {}
        x9_t, x0, sz = x9_first, 0, 2
        for b in range(B_LOC + DEPTH):
            if b < B_LOC:
                if b == x0 + sz:
                    x0, sz = b, min(C_IMG, B_LOC - b)
                    x9_t = x9_pool.tile([9, sz, NPIX], _BF16, tag="x9")
                    nc.sync.dma_start(x9_t[:], x9_d[:, x0:x0 + sz, :])
                hps[b] = conv1(b, x9_t, b - x0)
            if b >= DEPTH:
                conv2(b - DEPTH, hps.pop(b - DEPTH))

        nc.sync.dma_start(
            fcw_t[:],
            fcw_d[:].rearrange("p (o k g n) -> p o k g n", o=2, k=KB, g=NGRP))

        # fc: out[n, b] += fcw[:, ocb, k, g, :].T @ pT_all[:, ocb, k, g, :]
        fc_ps = psfc_pool.tile([NCLS, B_LOC], _F32)
        nmm = 2 * KB * NGRP
        i = 0
        for ocb in range(2):
            for k in range(KB):
                for g in range(NGRP):
                    nc.tensor.matmul(
                        fc_ps[:],
                        lhsT=fcw_t[:, ocb, k, g, :],
                        rhs=pT_all[:, ocb, k, g, :],
                        start=(i == 0), stop=(i == nmm - 1),
                    )
                    i += 1
        out_sb = consts.tile([NCLS, B_LOC], _F32)
        nc.vector.tensor_scalar_add(out_sb[:], fc_ps[:], fcb_t[:])
        nc.sync.dma_start(out_d[:].rearrange("b n -> n b"), out_sb[:])

    if not nc.is_finalized():
        nc.finalize()
    return nc


_NC_CACHE = {}


def _get_nc():
    key = "flip"
    if key not in _NC_CACHE:
        _NC_CACHE[key] = build_bass()
    return _NC_CACHE[key]


def _run(x, base_weight, w2, fc_w, fc_b, **spmd_kwargs):
    x = np.asarray(x, np.float32)
    base_weight = np.asarray(base_weight, np.float32)
    w2 = np.asarray(w2, np.float32)
    fc_w = np.asarray(fc_w, np.float32)
    fc_b = np.asarray(fc_b, np.float32)

    prep = _host_prep(x, base_weight, w2, fc_w, fc_b)
    nc = _get_nc()
    in_maps = []
    for i in range(N_CORES):
        m = dict(prep)
        m["x9"] = np.ascontiguousarray(prep["x9"][:, i * B_LOC:(i + 1) * B_LOC, :])
        in_maps.append(m)
    res = run_bass_kernel_spmd(nc, in_maps, list(range(N_CORES)), **spmd_kwargs)
    out = np.concatenate([res.results[i]["out"] for i in range(N_CORES)], axis=0)
    return out, res


def kernel(x, base_weight, w2, fc_w, fc_b):
    out, _ = _run(x, base_weight, w2, fc_w, fc_b)
    return out
